# revision 3
# baseline (speedup 1.0000x reference)
"""GNN scatter-mean + Linear kernel for Trainium2, 8 NeuronCores.

Strategy (node-sharded, no collectives):
  - CPU: sort edges by destination node, bucket them per core (each core owns
    1250 contiguous nodes = 10 tiles of 128), pre-scale each edge row by
    1/max(count[dst],1) so the device segment-sum directly yields the mean.
  - Device (per core): for each 128-node tile, accumulate its (sorted,
    padded-to-128) edge tiles into PSUM via one-hot matmuls on the PE
    (one-hot built on DVE from an iota + per-partition is_equal), then
    transpose the [128n, 256f] mean via PE and apply the 256x256 Linear
    (2 K-chunk matmuls against W.T), add bias, DMA out.
  - CPU: concatenate the 8 per-core [1250, 256] blocks.

Everything data-dependent about the *program* (slot counts per node tile) is
computed from the actual edge_index before tracing, and shared across all 8
cores so a single SPMD program serves every core.
"""

import sys

sys.path.insert(0, "/opt/trn_rl_repo")

from contextlib import ExitStack

import ml_dtypes
import numpy as np

N_NODES = 10000
N_EDGES = 320000
FEAT = 256
NCORES = 8
P = 128
NPC = (N_NODES + NCORES - 1) // NCORES  # 1250 nodes per core
NTILES = (NPC + P - 1) // P  # 10 node tiles per core
LAST_ROWS = NPC - (NTILES - 1) * P  # 98 real nodes in the last tile
CH = 16  # src slots per DMA chunk (16 * 128 * 256 * 2B = 1 MiB)

USE_BF16 = True

_bf16 = ml_dtypes.bfloat16


def _plan(dst):
    """Compute the shared program structure from the destination indices.

    Returns (perm, bounds, k_slots, base, nslot) where perm sorts edges by
    dst, bounds[c*NTILES + t] is the start of (core c, tile t)'s edge range in
    sorted order, and k_slots[t] is the number of 128-edge slots allotted to
    node tile t on EVERY core (max over cores, >= 1).
    """
    perm = np.argsort(dst, kind="stable")
    dst_sorted = dst[perm]
    edges_of = []  # (c, t) -> (lo, hi) in sorted order
    for c in range(NCORES):
        for t in range(NTILES):
            n0 = c * NPC + t * P
            n1 = min(c * NPC + min((t + 1) * P, NPC), N_NODES)
            lo = np.searchsorted(dst_sorted, n0, side="left")
            hi = np.searchsorted(dst_sorted, n1, side="left")
            edges_of.append((int(lo), int(hi)))
    k_slots = []
    for t in range(NTILES):
        mx = 1
        for c in range(NCORES):
            lo, hi = edges_of[c * NTILES + t]
            mx = max(mx, -(-(hi - lo) // P))
        k_slots.append(mx)
    base = np.concatenate([[0], np.cumsum(k_slots)])
    return perm, edges_of, k_slots, base, int(base[-1])


def _build_program(k_slots, chunk_sizes, nslot):
    from concourse import bacc, mybir
    import concourse.tile as tile

    f32 = mybir.dt.float32
    dt_c = mybir.dt.bfloat16 if USE_BF16 else mybir.dt.float32
    eq = mybir.AluOpType.is_equal
    add = mybir.AluOpType.add

    nc = bacc.Bacc("TRN2", target_bir_lowering=False, debug=False)

    src_drams = [
        nc.dram_tensor(f"src{i}", [P, ch, FEAT], dt_c, kind="ExternalInput")
        for i, ch in enumerate(chunk_sizes)
    ]
    dstrel_d = nc.dram_tensor("dstrel", [P, nslot], f32, kind="ExternalInput")
    wt_d = nc.dram_tensor("wt", [P, 2, FEAT], dt_c, kind="ExternalInput")
    bias_d = nc.dram_tensor("bias", [P, FEAT], f32, kind="ExternalInput")
    iota_d = nc.dram_tensor("iota", [P, P], dt_c, kind="ExternalInput")
    ident_d = nc.dram_tensor("ident", [P, P], dt_c, kind="ExternalInput")
    out_d = nc.dram_tensor("out", [NTILES, P, FEAT], f32, kind="ExternalOutput")

    base = [0]
    for k in k_slots:
        base.append(base[-1] + k)

    with tile.TileContext(nc) as tc, ExitStack() as ctx:
        const = ctx.enter_context(tc.tile_pool(name="const", bufs=1))
        srcp = ctx.enter_context(tc.tile_pool(name="srcp", bufs=3))
        ohp = ctx.enter_context(tc.tile_pool(name="ohp", bufs=8))
        meanp = ctx.enter_context(tc.tile_pool(name="meanp", bufs=2))
        mtp = ctx.enter_context(tc.tile_pool(name="mtp", bufs=2))
        outp = ctx.enter_context(tc.tile_pool(name="outp", bufs=2))
        ps_agg = ctx.enter_context(tc.tile_pool(name="ps_agg", bufs=2, space="PSUM"))
        ps_t = ctx.enter_context(tc.tile_pool(name="ps_t", bufs=2, space="PSUM"))
        ps_out = ctx.enter_context(tc.tile_pool(name="ps_out", bufs=2, space="PSUM"))

        dstrel_sb = const.tile([P, nslot], f32)
        nc.sync.dma_start(dstrel_sb[:], dstrel_d[:])
        wt_sb = const.tile([P, 2, FEAT], dt_c)
        nc.sync.dma_start(wt_sb[:], wt_d[:])
        bias_sb = const.tile([P, FEAT], f32)
        nc.sync.dma_start(bias_sb[:], bias_d[:])
        iota_sb = const.tile([P, P], dt_c)
        nc.sync.dma_start(iota_sb[:], iota_d[:])
        ident_sb = const.tile([P, P], dt_c)
        nc.sync.dma_start(ident_sb[:], ident_d[:])

        chunk_tiles = [None] * len(chunk_sizes)

        def get_chunk(ci):
            if chunk_tiles[ci] is None:
                ct = srcp.tile([P, chunk_sizes[ci], FEAT], dt_c, tag="src_chunk")
                nc.sync.dma_start(ct[:], src_drams[ci][:])
                chunk_tiles[ci] = ct
            return chunk_tiles[ci]

        for t in range(NTILES):
            agg = ps_agg.tile([P, FEAT], f32)
            kst = k_slots[t]
            for k in range(kst):
                s = base[t] + k
                ci, cl = divmod(s, CH)
                ct = get_chunk(ci)
                oh = ohp.tile([P, P], dt_c)
                nc.vector.tensor_scalar(
                    oh[:], iota_sb[:], dstrel_sb[:, s : s + 1], None, eq
                )
                nc.tensor.matmul(
                    agg[:],
                    oh[:],
                    ct[:, cl, :],
                    start=(k == 0),
                    stop=(k == kst - 1),
                )
            mean = meanp.tile([P, FEAT], dt_c)
            nc.any.tensor_copy(mean[:], agg[:])
            tp = ps_t.tile([P, 2, P], dt_c)
            nc.tensor.transpose(tp[:, 0, :], mean[:, 0:P], ident_sb[:])
            nc.tensor.transpose(tp[:, 1, :], mean[:, P : 2 * P], ident_sb[:])
            mt = mtp.tile([P, 2, P], dt_c)
            nc.any.tensor_copy(mt[:], tp[:])
            op_ = ps_out.tile([P, FEAT], f32)
            nc.tensor.matmul(op_[:], mt[:, 0, :], wt_sb[:, 0, :], start=True, stop=False)
            nc.tensor.matmul(op_[:], mt[:, 1, :], wt_sb[:, 1, :], start=False, stop=True)
            ob = outp.tile([P, FEAT], f32)
            nc.vector.tensor_tensor(ob[:], op_[:], bias_sb[:], op=add)
            nc.sync.dma_start(out_d[t], ob[:])

    nc.compile()
    return nc


def _prepare(inputs):
    """CPU-side sharding: returns (nc, in_maps) ready for SPMD dispatch."""
    src = np.asarray(inputs["source_node_representation_with_coefficient"])
    edge_index = np.asarray(inputs["edge_index"])
    W = np.asarray(inputs["W"], dtype=np.float32)
    b = np.asarray(inputs["b"], dtype=np.float32)
    assert src.shape == (N_EDGES, FEAT) and edge_index.shape == (2, N_EDGES)

    np_c = _bf16 if USE_BF16 else np.float32

    dst = edge_index[1].astype(np.int64)
    counts = np.bincount(dst, minlength=N_NODES)
    recip = (1.0 / np.maximum(counts, 1)).astype(np.float32)
    rec_edge = recip[dst]

    perm, edges_of, k_slots, base, nslot = _plan(dst)

    nchunk = -(-nslot // CH)
    chunk_sizes = [min(CH, nslot - i * CH) for i in range(nchunk)]

    nc = _build_program(k_slots, chunk_sizes, nslot)

    # shared (replicated) small tensors
    wt_packed = np.ascontiguousarray(
        W.T.reshape(2, P, FEAT).transpose(1, 0, 2)
    ).astype(np_c)
    bias_tile = np.ascontiguousarray(np.broadcast_to(b, (P, FEAT))).astype(np.float32)
    iota_tile = np.ascontiguousarray(
        np.broadcast_to(np.arange(P, dtype=np.float32), (P, P))
    ).astype(np_c)
    ident_tile = np.eye(P, dtype=np.float32).astype(np_c)

    in_maps = []
    for c in range(NCORES):
        pos = np.full((nslot, P), -1, dtype=np.int64)
        tile_off = np.zeros((nslot,), dtype=np.int64)
        for t in range(NTILES):
            lo, hi = edges_of[c * NTILES + t]
            n = hi - lo
            b0 = base[t]
            if n > 0:
                flat = pos[b0 : b0 + -(-n // P)].reshape(-1)
                flat[:n] = perm[lo:hi]
            tile_off[b0 : base[t + 1]] = c * NPC + t * P

        valid = pos >= 0
        rel = np.where(valid, dst[pos] - tile_off[:, None], 0)
        assert rel.min() >= 0 and rel.max() < P
        scale = np.where(valid, rec_edge[pos], 0.0).astype(np.float32)

        # gather + pre-scale all of this core's edge rows, in slot order
        srcg = src[pos.reshape(-1)] * scale.reshape(-1, 1)
        srcg = srcg.astype(np_c)

        m = {
            "dstrel": np.ascontiguousarray(rel.T.astype(np.float32)),
            "wt": wt_packed,
            "bias": bias_tile,
            "iota": iota_tile,
            "ident": ident_tile,
        }
        for i, ch in enumerate(chunk_sizes):
            s0 = i * CH
            blk = srcg[s0 * P : (s0 + ch) * P].reshape(ch, P, FEAT)
            m[f"src{i}"] = np.ascontiguousarray(blk.transpose(1, 0, 2))
        in_maps.append(m)

    return nc, in_maps


def _gather_output(results):
    blocks = []
    for c in range(NCORES):
        o = np.asarray(results[c]["out"], dtype=np.float32)  # [NTILES, P, FEAT]
        o = o.reshape(NTILES * P, FEAT)[:NPC]
        blocks.append(o)
    return np.concatenate(blocks, axis=0)[:N_NODES]


def run(inputs, trace=False, **spmd_kwargs):
    from concourse.bass_utils import run_bass_kernel_spmd

    nc, in_maps = _prepare(inputs)
    res = run_bass_kernel_spmd(
        nc, in_maps, core_ids=list(range(NCORES)), trace=trace, **spmd_kwargs
    )
    return _gather_output(res.results), res


def kernel(**inputs) -> np.ndarray:
    out, _ = run(inputs, trace=False)
    return out
